# revision 26
# baseline (speedup 1.0000x reference)
"""BitLinear (absmean ternary-quantized linear) on 8 TRN2 NeuronCores.

Strategy (tensor-parallel, column sharding, fp8 DoubleRow matmul):
  - weight [16384, 4096] sharded along out-features: 2048 rows per core.
  - absmean scale is global over W: each core computes a per-partition abs-sum
    of its shard, AllReduce(add) across the 8 cores, then a ones-matmul
    reduces across partitions and broadcasts the global sum. The absum input
    is streamed as fp8(64*w) - 64 maps the kaiming bound into e4m3's normal
    range, RNE keeps the mean unbiased to ~1e-5 relative, and the stream is
    4x smaller than fp32 (the head phase is DMA-bound at ~180 GB/s).
  - quantize: wq = (w > T) - (w < -T) with T = 0.5*scale from the fp32 W
    (equivalent to clip(round(w/scale), -1, 1)); stored as fp8e4 {-1,0,1}
    (exact), unscaled; the fp32 scale is applied in the PSUM->SBUF copy.
  - matmul: fp8e4 DoubleRow perf mode - each instruction contracts K=256
    (two 128-row planes) at double rate: lhsT = x tile [128, 2, 128]
    stationary, rhs = wq slice [128, 2, 512] moving, fp32 PSUM accumulate.
  - precision: e4m3 x alone gives rel err ~0.024 (> 2e-2 gate). A residual
    pass r8 = e4m3(x - e4m3(x)) over the first 6/16 of K cancels that much
    of the quantization noise power -> rel err ~0.019 at 1.375x matmul cost,
    still well ahead of bf16 (2x the PE work of the fp8 main pass).
  - the AllReduce is fabric-sensitive: concurrent bulk DMA inflates its
    latency ~7x. The fp32 W stream for the quantizer therefore rides the
    same gpsimd ring as the collective bounces, queued BEHIND the readback,
    so the fabric is quiet during the collective by construction.
  - pass 1 (nb=0) starts while the quantizer is still streaming: the first
    8 m-blocks are wave-interleaved (4 k-chunks across all 8 blocks per
    wave, PSUM groups held open) so the PE tracks the quant stream front
    instead of serializing behind the last chunk of block 0. Pass 2 runs
    nb=1..3 per m-block (stationary x shared by the three nb matmuls).
  - out is stored bf16 (halves the store traffic; adds ~1e-4 to rel err),
    widened to fp32 on the host.
  - rings: sync = absum share + x/r stream; gpsimd = absum share, collective
    bounces, fp32 W stream; scalar = absum share + out stores. Engines:
    DVE = absum reduces + quant math; Activation = absum Abs+accum share +
    scaled PSUM evacuation.
"""

import os
import sys

import numpy as np

sys.path.insert(0, "/opt/trn_rl_repo")

import ml_dtypes  # noqa: E402

from concourse import bacc, mybir, tile  # noqa: E402
from concourse.bass_utils import run_bass_kernel_spmd  # noqa: E402


def _install_ntff_hook_shim():
    """bass_utils' trace path needs antenv.axon_hooks, which this image's
    antenv lacks. Recreate the boot-time hook against the axon PJRT .so so
    NTFF profiling (HW exec_time_ns) works."""
    import contextlib
    import ctypes
    import types

    try:
        from antenv.axon_hooks import get_axon_ntff_profile_hook  # noqa: F401

        return  # real module present
    except ImportError:
        pass

    so_path = "/opt/axon/libaxon_pjrt.so"
    if not os.path.exists(so_path):
        return
    lib = ctypes.CDLL(so_path)
    if not hasattr(lib, "axon_start_nrt_profile"):
        return
    lib.axon_start_nrt_profile.argtypes = [
        ctypes.POINTER(ctypes.c_int64),
        ctypes.c_size_t,
    ]
    lib.axon_start_nrt_profile.restype = ctypes.c_int64
    lib.axon_stop_nrt_profile.argtypes = [ctypes.c_char_p]
    lib.axon_stop_nrt_profile.restype = ctypes.c_int64

    @contextlib.contextmanager
    def _hook(output_dir, device_ids):
        import jax

        jax.devices()
        if device_ids:
            ids = (ctypes.c_int64 * len(device_ids))(*device_ids)
            rc = lib.axon_start_nrt_profile(ids, len(device_ids))
        else:
            rc = lib.axon_start_nrt_profile(None, 0)
        if rc != 0:
            raise RuntimeError(f"axon_start_nrt_profile rc={rc}")
        try:
            yield
        finally:
            n = lib.axon_stop_nrt_profile(str(output_dir).encode())
            if n < 0:
                raise RuntimeError(f"axon_stop_nrt_profile rc={n}")

    mod = types.ModuleType("antenv.axon_hooks")
    _state = {"hook": _hook}
    mod.set_axon_ntff_profile_hook = lambda h: _state.__setitem__("hook", h)
    mod.get_axon_ntff_profile_hook = lambda: _state["hook"]
    sys.modules["antenv.axon_hooks"] = mod


_install_ntff_hook_shim()

N_CORES = 8
B, S, K, NF = 4, 2048, 4096, 16384
M = B * S  # 8192 tokens
NL = NF // N_CORES  # 2048 out-features per core
KT = K // 128  # 32 contraction subtiles of 128
K2 = KT // 2  # 16 DoubleRow chunks of 256
MB = M // 128  # 64 token blocks
NB = NL // 512  # 4 out-feature chunks of 512
K2R = 6  # residual DoubleRow chunks (first 6/16 of K; rel err ~0.019)
KR = 2 * K2R  # residual subtiles (k < 1536)
NJ = K2 + K2R  # matmuls per (mb, nb) accumulation group (22)
AT = 8  # absum stream tiles [128, 8192] fp8 (8KB descriptors - the DMA
#   rings are descriptor-rate-bound, so fat per-partition rows matter)
ASL = 4  # absum compute slices per tile ([128, 2048] each)
QG = KT // 4  # quant quad-groups per nb (8)
NQ = NB * QG  # total quant quad tiles (32)
WAVED = 6  # wave-interleaved m-blocks at the start of pass 1
INV_NELEM = 1.0 / (NF * K * 64.0)  # absum stream carries 64*w

LAST_EXEC_NS = None
LAST_RESULTS = None

_nc_cache = None


def _build_nc():
    f32 = mybir.dt.float32
    bf16 = mybir.dt.bfloat16
    f8 = mybir.dt.float8e4

    nc = bacc.Bacc(
        "TRN2", target_bir_lowering=False, debug=False, num_devices=N_CORES
    )
    xs = nc.declare_dram_parameter("xs", [MB, 128, KT, 128], f8, isOutput=False)
    rs = nc.declare_dram_parameter("rs", [MB, 128, KR, 128], f8, isOutput=False)
    wa = nc.declare_dram_parameter("wa", [AT, 128, 8192], f8, isOutput=False)
    wt = nc.declare_dram_parameter("wt", [NB, 128, KT, 512], f32, isOutput=False)
    out = nc.declare_dram_parameter("out", [M, NL], bf16, isOutput=True)

    add = mybir.AluOpType.add
    mult = mybir.AluOpType.mult
    sub = mybir.AluOpType.subtract
    amax = mybir.AluOpType.max
    dr = mybir.MatmulPerfMode.DoubleRow

    with tile.TileContext(nc) as tc:
        with (
            tc.tile_pool(name="wq_pool", bufs=1) as wq_pool,
            tc.tile_pool(name="astage", bufs=4) as astage,
            tc.tile_pool(name="wstage", bufs=4) as wstage,
            tc.tile_pool(name="tmp_pool", bufs=2) as tmp_pool,
            tc.tile_pool(name="xstage", bufs=7) as xstage,
            tc.tile_pool(name="rstage", bufs=7) as rstage,
            tc.tile_pool(name="ostage", bufs=6) as ostage,
            tc.tile_pool(name="small", bufs=1) as small,
            tc.tile_pool(name="psum", bufs=8, space="PSUM") as psum_pool,
            tc.tile_pool(name="dram", bufs=1, space="DRAM") as dram_pool,
        ):
            # Resident quantized weights, one tile per nb so pass-1 reads of
            # the nb=0 tile never false-depend on quant writes to nb=1..3.
            # Each is [128(k), kt, 512], sliced [:, 2j:2j+2, :] as the
            # DoubleRow moving operand.
            wq = {
                nb: wq_pool.tile([128, KT, 512], f8, name=f"wq{nb}", tag=f"wq{nb}")
                for nb in range(NB)
            }

            # ---- Phase A absum: fp8(64*w) streamed round-robin over all
            # three free DMA rings; abs-sums split between DVE
            # (tensor_reduce) and the Activation engine (Abs + accum).
            # Software-pipelined: DMA lookahead stays below the pool depth so
            # ring-buffer reuse always lands behind an already-emitted reader.
            ALOOK = 3
            atiles = {}
            arings = (nc.sync, nc.gpsimd, nc.scalar)

            def a_issue(t):
                wab = astage.tile([128, 8192], f8, name="wab", tag="wab")
                arings[t % 3].dma_start(wab[:], wa[t])
                atiles[t] = wab

            trash = small.tile([128, 2048], bf16, name="trash")
            partials = small.tile([128, AT * ASL], f32, name="partials")

            def a_consume(s):
                # consume slice s of tile s // ASL; alternate DVE / Activation
                wab = atiles[s // ASL]
                sl = s % ASL
                view = wab[:, sl * 2048 : (sl + 1) * 2048]
                if s % 2 == 0:
                    nc.vector.tensor_reduce(
                        partials[:, s : s + 1],
                        view,
                        axis=mybir.AxisListType.X,
                        op=add,
                        apply_absolute_value=True,
                    )
                else:
                    nc.scalar.activation(
                        trash[:],
                        view,
                        mybir.ActivationFunctionType.Abs,
                        accum_out=partials[:, s : s + 1],
                    )

            for t in range(ALOOK):
                a_issue(t)
            for s in range(AT * ASL):
                if s % ASL == 0:
                    t = s // ASL
                    if t + ALOOK < AT:
                        a_issue(t + ALOOK)
                a_consume(s)
            atiles.clear()

            loc = small.tile([128, 1], f32, name="loc")
            nc.vector.tensor_reduce(
                loc[:], partials[:], axis=mybir.AxisListType.X, op=add
            )
            # Gate the x/r stream (emitted below, same sync ring) behind the
            # local abs-sum: its transfers otherwise steal fabric bandwidth
            # from the phase-A stream, which is the AllReduce's critical path
            # on every core.
            xgate = small.tile([128, 1], f32, name="xgate")
            nc.sync.dma_start(xgate[:], loc[:])
            # Bounce DMAs + collective all on gpsimd: the in-order queue
            # naturally sequences write -> AllReduce -> readback, and the
            # fp32 W stream below is queued BEHIND the readback on this same
            # ring, keeping the fabric quiet while the collective runs.
            cc_in = dram_pool.tile([128, 1], f32, name="cc_in")
            cc_out = dram_pool.tile([128, 1], f32, name="cc_out", addr_space="Shared")
            nc.gpsimd.dma_start(cc_in[:], loc[:])
            with tc.high_priority():
                nc.gpsimd.collective_compute(
                    "AllReduce",
                    add,
                    replica_groups=[list(range(N_CORES))],
                    ins=[cc_in.opt()],
                    outs=[cc_out.opt()],
                )
            ar_sb = small.tile([128, 1], f32, name="ar_sb")
            nc.gpsimd.dma_start(ar_sb[:], cc_out[:])

            # fp32 W stream for the quantizer, nb-major, on the gpsimd ring
            # (behind the collective readback).
            WLOOK = 2
            wtiles = {}

            def w_issue(i):
                nb, g = divmod(i, QG)
                wst = wstage.tile([128, 4, 512], f32, name="wst", tag="wst")
                nc.gpsimd.dma_start(wst[:], wt[nb, :, 4 * g : 4 * g + 4, :])
                wtiles[i] = wst

            for i in range(WLOOK):
                w_issue(i)

            # Reduce across partitions + broadcast: ones[128,128].T @ ar_sb
            ones = small.tile([128, 128], f32, name="ones")
            nc.vector.memset(ones[:], 1.0)
            psum_s = psum_pool.tile([128, 1], f32, name="psum_s", tag="mm")
            nc.tensor.matmul(psum_s[:], ones[:], ar_sb[:], start=True, stop=True)

            scale_sb = small.tile([128, 1], f32, name="scale_sb")
            nc.vector.tensor_scalar(
                out=scale_sb[:], in0=psum_s[:],
                scalar1=INV_NELEM, scalar2=1e-5, op0=mult, op1=amax,
            )
            thr = small.tile([128, 1], f32, name="thr")
            nc.vector.tensor_scalar(
                out=thr[:], in0=scale_sb[:], scalar1=0.5, scalar2=None, op0=mult
            )
            nthr = small.tile([128, 1], f32, name="nthr")
            nc.vector.tensor_scalar(
                out=nthr[:], in0=scale_sb[:], scalar1=-0.5, scalar2=None, op0=mult
            )

            # ---- Phase B: quantize fp32 W -> wq in {-1,0,1} fp8, nb-major so
            # pass 1 (nb=0) can start after a quarter of W is quantized.
            def w_consume(i):
                nb, g = divmod(i, QG)
                wst = wtiles.pop(i)
                t1 = tmp_pool.tile([128, 4, 512], f32, name="t1", tag="t1")
                # t1 = (w < -T)
                nc.vector.tensor_scalar(
                    out=t1[:], in0=wst[:],
                    scalar1=nthr[:], scalar2=None,
                    op0=mybir.AluOpType.is_lt,
                )
                # wq = (w > T) - t1
                nc.vector.scalar_tensor_tensor(
                    out=wq[nb][:, 4 * g : 4 * g + 4, :],
                    in0=wst[:],
                    scalar=thr[:], in1=t1[:],
                    op0=mybir.AluOpType.is_gt, op1=sub,
                )

            for i in range(NQ):
                if i + WLOOK < NQ:
                    w_issue(i + WLOOK)
                w_consume(i)

            # ---- Phase C: out[mb] = x[mb] @ wq.T, fp8 DoubleRow ----
            def x_issue(mb):
                xst = xstage.tile([128, KT, 128], f8, name="xst", tag="xst")
                nc.sync.dma_start(xst[:, :, :], xs[mb])
                rst = rstage.tile([128, KR, 128], f8, name="rst", tag="rst")
                nc.sync.dma_start(rst[:, :, :], rs[mb])
                return xst, rst

            def lhs_of(xt, j):
                xst, rst = xt
                if j < K2:
                    return xst[:, 2 * j : 2 * j + 2, :]
                jj = j - K2
                return rst[:, 2 * jj : 2 * jj + 2, :]

            def mm(psum, xt, j, nb):
                jj = j if j < K2 else j - K2
                nc.tensor.matmul(
                    psum[:],
                    lhs_of(xt, j),
                    wq[nb][:, 2 * jj : 2 * jj + 2, :],
                    start=(j == 0),
                    stop=(j == NJ - 1),
                    perf_mode=dr,
                )

            def evac(mb, nb, psum):
                ost = ostage.tile([128, 512], bf16, name="ost", tag="ost")
                # out = psum * scale, on ScalarE (has a PSUM port), bf16 store
                nc.scalar.activation(
                    ost[:],
                    psum[:],
                    mybir.ActivationFunctionType.Copy,
                    scale=scale_sb[:],
                )
                nc.scalar.dma_start(
                    out[mb * 128 : (mb + 1) * 128, nb * 512 : (nb + 1) * 512],
                    ost[:],
                )

            def evac2(mb, nb, ps_a, ps_b):
                # two adjacent nb chunks into one [128,1024] store: the store
                # queues are descriptor-rate-bound, so 2 KiB rows beat 1 KiB
                ost = ostage.tile([128, 1024], bf16, name="ost2", tag="ost2")
                nc.scalar.activation(
                    ost[:, 0:512],
                    ps_a[:],
                    mybir.ActivationFunctionType.Copy,
                    scale=scale_sb[:],
                )
                nc.scalar.activation(
                    ost[:, 512:1024],
                    ps_b[:],
                    mybir.ActivationFunctionType.Copy,
                    scale=scale_sb[:],
                )
                nc.scalar.dma_start(
                    out[mb * 128 : (mb + 1) * 128, nb * 512 : (nb + 2) * 512],
                    ost[:],
                )

            def do_block(mb, nbs, xt):
                psums = {
                    nb: psum_pool.tile([128, 512], f32, name=f"ps_{mb}_{nb}", tag="mm")
                    for nb in nbs
                }
                for j in range(NJ):
                    for nb in nbs:
                        mm(psums[nb], xt, j, nb)
                if nbs == [1, 2, 3]:
                    evac2(mb, 1, psums[1], psums[2])
                    evac(mb, 3, psums[3])
                else:
                    for nb in nbs:
                        evac(mb, nb, psums[nb])

            # Pass 1, wave-interleaved prefix: 8 blocks advance 4 k-chunks at
            # a time so the PE tracks the nb=0 quant stream front.
            xts = {}
            for mb in range(WAVED):
                xts[(0, mb)] = x_issue(mb)
            wpsums = [
                psum_pool.tile([128, 512], f32, name=f"wps_{mb}", tag="mm")
                for mb in range(WAVED)
            ]
            for w0 in range(0, NJ, 4):
                for mb in range(WAVED):
                    for j in range(w0, min(w0 + 4, NJ)):
                        mm(wpsums[mb], xts[(0, mb)], j, 0)
            for mb in range(WAVED):
                xts.pop((0, mb))
                evac(mb, 0, wpsums[mb])

            # Pass 1 remainder + pass 2, with x/r prefetched XLOOK blocks out.
            XLOOK = 4
            schedule = [(0, mb, [0]) for mb in range(WAVED, MB)]
            schedule += [(1, mb, [1, 2, 3]) for mb in range(MB)]
            for idx, (p, mb, nbs) in enumerate(schedule):
                if idx == 0:
                    for p2, mb2, _ in schedule[: XLOOK + 1]:
                        xts[(p2, mb2)] = x_issue(mb2)
                look = idx + XLOOK + 1
                if look < len(schedule):
                    p2, mb2, _ = schedule[look]
                    xts[(p2, mb2)] = x_issue(mb2)
                do_block(mb, nbs, xts.pop((p, mb)))

    nc.compile()
    return nc


def _get_nc():
    global _nc_cache
    if _nc_cache is None:
        _nc_cache = _build_nc()
    return _nc_cache


def kernel(x: np.ndarray, weight: np.ndarray) -> np.ndarray:
    global LAST_EXEC_NS, LAST_RESULTS
    x = np.asarray(x, dtype=np.float32)
    weight = np.asarray(weight, dtype=np.float32)

    nc = _get_nc()

    f8 = ml_dtypes.float8_e4m3

    # x -> stationary tile layout [mb, k(part), kt, m]: per (mb, p) the
    # [kt, m] plane is contiguous, so each m-block loads as one DMA.
    xf = x.reshape(M, K)
    x8 = xf.astype(f8)
    xsh = np.ascontiguousarray(
        x8.reshape(MB, 128, KT, 128).transpose(0, 3, 2, 1)
    )
    # residual of the fp8 cast, itself in fp8, for the first 6/16 of K
    r = (xf - x8.astype(np.float32))[:, : KR * 128].astype(f8)
    rsh = np.ascontiguousarray(r.reshape(MB, 128, KR, 128).transpose(0, 3, 2, 1))

    in_maps = []
    for c in range(N_CORES):
        wsh = weight[c * NL : (c + 1) * NL, :]  # [2048, 4096] fp32
        # quant layout [nb, k(part), kt, n]: per partition the (kt, n) plane
        # is contiguous, so a kt-group loads as one descriptor per partition.
        wtc = np.ascontiguousarray(
            wsh.T.reshape(KT, 128, NB, 512).transpose(2, 1, 0, 3)
        )
        # absum stream: fp8(64*w); any partitioning works for a plain sum
        wac = (wsh * np.float32(64.0)).astype(f8).reshape(AT, 128, 8192)
        in_maps.append({"xs": xsh, "rs": rsh, "wa": wac, "wt": wtc})

    # Warm the per-device PJRT dispatch path (device contexts, executable
    # load machinery) with a trivial sharded op so the real launch below
    # starts the 8 cores with minimal stagger - the in-kernel AllReduce
    # otherwise turns launch skew into idle time on every early core.
    import jax
    from jax.sharding import Mesh, NamedSharding, PartitionSpec

    devs = jax.devices()[:N_CORES]
    wmesh = Mesh(np.asarray(devs), ("core",))
    warm = jax.device_put(
        np.zeros((N_CORES, 128), np.float32),
        NamedSharding(wmesh, PartitionSpec("core")),
    )
    jax.jit(lambda t: t + 1.0)(warm).block_until_ready()

    trace = bool(int(os.environ.get("BASS_KERNEL_TRACE", "0")))
    res = run_bass_kernel_spmd(
        nc, in_maps, core_ids=list(range(N_CORES)), trace=trace
    )
    LAST_EXEC_NS = res.exec_time_ns
    LAST_RESULTS = res

    outs = [np.asarray(res.results[c]["out"]) for c in range(N_CORES)]
    full = (
        np.concatenate(outs, axis=1).astype(np.float32).reshape(B, S, NF)
    )
    return full


# revision 34
# speedup vs baseline: 1.0326x; 1.0326x over previous
"""BitLinear (absmean ternary-quantized linear) on 8 TRN2 NeuronCores.

Strategy (tensor-parallel, column sharding, fp8 DoubleRow matmul):
  - weight [16384, 4096] sharded along out-features: 2048 rows per core.
  - absmean scale is global over W: each core computes a per-partition abs-sum
    of its shard, AllReduce(add) across the 8 cores, then a ones-matmul
    reduces across partitions and broadcasts the global sum. The absum input
    is streamed as fp8(64*w) - 64 maps the kaiming bound into e4m3's normal
    range, RNE keeps the mean unbiased to ~1e-5 relative, and the stream is
    4x smaller than fp32 (the head phase is DMA-bound at ~180 GB/s).
  - quantize: wq = (w > T) - (w < -T) with T = 0.5*scale from the fp32 W
    (equivalent to clip(round(w/scale), -1, 1)); stored as fp8e4 {-1,0,1}
    (exact), unscaled; the fp32 scale is applied in the PSUM->SBUF copy.
  - matmul: fp8e4 DoubleRow perf mode - each instruction contracts K=256
    (two 128-row planes) at double rate: lhsT = x tile [128, 2, 128]
    stationary, rhs = wq slice [128, 2, 512] moving, fp32 PSUM accumulate.
  - precision: e4m3 x alone gives rel err ~0.024 (> 2e-2 gate). A residual
    pass r8 = e4m3(x - e4m3(x)) over the first 6/16 of K cancels that much
    of the quantization noise power -> rel err ~0.019 at 1.375x matmul cost,
    still well ahead of bf16 (2x the PE work of the fp8 main pass).
  - the AllReduce is fabric-sensitive: concurrent bulk DMA inflates its
    latency ~7x. The fp32 W stream for the quantizer therefore rides the
    same gpsimd ring as the collective bounces, queued BEHIND the readback,
    so the fabric is quiet during the collective by construction.
  - pass 1 (nb=0) starts while the quantizer is still streaming: the first
    8 m-blocks are wave-interleaved (4 k-chunks across all 8 blocks per
    wave, PSUM groups held open) so the PE tracks the quant stream front
    instead of serializing behind the last chunk of block 0. Pass 2 runs
    nb=1..3 per m-block (stationary x shared by the three nb matmuls).
  - out is stored bf16 (halves the store traffic; adds ~1e-4 to rel err),
    widened to fp32 on the host.
  - rings: sync = absum share + x/r stream; gpsimd = absum share, collective
    bounces, fp32 W stream; scalar = absum share + out stores. Engines:
    DVE = absum reduces + quant math; Activation = absum Abs+accum share +
    scaled PSUM evacuation.
"""

import os
import sys

import numpy as np

sys.path.insert(0, "/opt/trn_rl_repo")

import ml_dtypes  # noqa: E402

from concourse import bacc, mybir, tile  # noqa: E402
from concourse.bass_utils import run_bass_kernel_spmd  # noqa: E402


def _install_ntff_hook_shim():
    """bass_utils' trace path needs antenv.axon_hooks, which this image's
    antenv lacks. Recreate the boot-time hook against the axon PJRT .so so
    NTFF profiling (HW exec_time_ns) works."""
    import contextlib
    import ctypes
    import types

    try:
        from antenv.axon_hooks import get_axon_ntff_profile_hook  # noqa: F401

        return  # real module present
    except ImportError:
        pass

    so_path = "/opt/axon/libaxon_pjrt.so"
    if not os.path.exists(so_path):
        return
    lib = ctypes.CDLL(so_path)
    if not hasattr(lib, "axon_start_nrt_profile"):
        return
    lib.axon_start_nrt_profile.argtypes = [
        ctypes.POINTER(ctypes.c_int64),
        ctypes.c_size_t,
    ]
    lib.axon_start_nrt_profile.restype = ctypes.c_int64
    lib.axon_stop_nrt_profile.argtypes = [ctypes.c_char_p]
    lib.axon_stop_nrt_profile.restype = ctypes.c_int64

    @contextlib.contextmanager
    def _hook(output_dir, device_ids):
        import jax

        jax.devices()
        if device_ids:
            ids = (ctypes.c_int64 * len(device_ids))(*device_ids)
            rc = lib.axon_start_nrt_profile(ids, len(device_ids))
        else:
            rc = lib.axon_start_nrt_profile(None, 0)
        if rc != 0:
            raise RuntimeError(f"axon_start_nrt_profile rc={rc}")
        try:
            yield
        finally:
            n = lib.axon_stop_nrt_profile(str(output_dir).encode())
            if n < 0:
                raise RuntimeError(f"axon_stop_nrt_profile rc={n}")

    mod = types.ModuleType("antenv.axon_hooks")
    _state = {"hook": _hook}
    mod.set_axon_ntff_profile_hook = lambda h: _state.__setitem__("hook", h)
    mod.get_axon_ntff_profile_hook = lambda: _state["hook"]
    sys.modules["antenv.axon_hooks"] = mod


_install_ntff_hook_shim()

N_CORES = 8
B, S, K, NF = 4, 2048, 4096, 16384
M = B * S  # 8192 tokens
NL = NF // N_CORES  # 2048 out-features per core
KT = K // 128  # 32 contraction subtiles of 128
K2 = KT // 2  # 16 DoubleRow chunks of 256
MB = M // 128  # 64 token blocks
NB = NL // 512  # 4 out-feature chunks of 512
K2R = 6  # residual DoubleRow chunks (first 6/16 of K; rel err ~0.019)
KR = 2 * K2R  # residual subtiles (k < 1536)
NJ = K2 + K2R  # matmuls per (mb, nb) accumulation group (22)
AT = 32  # absum stream tiles [128, 2048] fp8
QG = KT // 4  # quant quad-groups per nb (8)
NQ = NB * QG  # total quant quad tiles (32)
WAVED = 8  # wave-interleaved m-blocks at the start of pass 1
INV_NELEM = 1.0 / (NF * K * 64.0)  # absum stream carries 64*w

LAST_EXEC_NS = None
LAST_RESULTS = None

_nc_cache = None


def _build_nc():
    f32 = mybir.dt.float32
    bf16 = mybir.dt.bfloat16
    f8 = mybir.dt.float8e4

    nc = bacc.Bacc(
        "TRN2", target_bir_lowering=False, debug=False, num_devices=N_CORES
    )
    xs = nc.declare_dram_parameter("xs", [MB, 128, KT, 128], f8, isOutput=False)
    rs = nc.declare_dram_parameter("rs", [MB, 128, KR, 128], f8, isOutput=False)
    wa = nc.declare_dram_parameter("wa", [AT, 128, 2048], f8, isOutput=False)
    wt = nc.declare_dram_parameter("wt", [NB, 128, KT, 512], f32, isOutput=False)
    out = nc.declare_dram_parameter("out", [M, NL], bf16, isOutput=True)

    add = mybir.AluOpType.add
    mult = mybir.AluOpType.mult
    sub = mybir.AluOpType.subtract
    amax = mybir.AluOpType.max
    dr = mybir.MatmulPerfMode.DoubleRow

    with tile.TileContext(nc) as tc:
        with (
            tc.tile_pool(name="wq_pool", bufs=1) as wq_pool,
            tc.tile_pool(name="astage", bufs=6) as astage,
            tc.tile_pool(name="wstage", bufs=6) as wstage,
            tc.tile_pool(name="tmp_pool", bufs=2) as tmp_pool,
            tc.tile_pool(name="xstage", bufs=9) as xstage,
            tc.tile_pool(name="rstage", bufs=9) as rstage,
            tc.tile_pool(name="ostage", bufs=6) as ostage,
            tc.tile_pool(name="small", bufs=1) as small,
            tc.tile_pool(name="psum", bufs=8, space="PSUM") as psum_pool,
            tc.tile_pool(name="dram", bufs=1, space="DRAM") as dram_pool,
        ):
            # Resident quantized weights, one tile per nb so pass-1 reads of
            # the nb=0 tile never false-depend on quant writes to nb=1..3.
            # Each is [128(k), kt, 512], sliced [:, 2j:2j+2, :] as the
            # DoubleRow moving operand.
            wq = {
                nb: wq_pool.tile([128, KT, 512], f8, name=f"wq{nb}", tag=f"wq{nb}")
                for nb in range(NB)
            }

            # ---- Phase A absum: fp8(64*w) streamed round-robin over all
            # three free DMA rings; abs-sums split between DVE
            # (tensor_reduce) and the Activation engine (Abs + accum).
            # Software-pipelined: DMA lookahead stays below the pool depth so
            # ring-buffer reuse always lands behind an already-emitted reader.
            ALOOK = 5
            atiles = {}
            arings = (nc.sync, nc.gpsimd, nc.scalar)

            def a_issue(t):
                wab = astage.tile([128, 2048], f8, name="wab", tag="wab")
                arings[t % 3].dma_start(wab[:], wa[t])
                atiles[t] = wab

            trash = small.tile([128, 2048], bf16, name="trash")
            trash3 = small.tile([128, 2048], bf16, name="trash3")
            partials = small.tile([128, AT], f32, name="partials")

            def a_consume(t):
                wab = atiles.pop(t)
                if t % 2 == 0:
                    nc.vector.tensor_reduce(
                        partials[:, t : t + 1],
                        wab[:],
                        axis=mybir.AxisListType.X,
                        op=add,
                        apply_absolute_value=True,
                    )
                else:
                    nc.scalar.activation(
                        trash[:],
                        wab[:],
                        mybir.ActivationFunctionType.Abs,
                        accum_out=partials[:, t : t + 1],
                    )

            for t in range(ALOOK):
                a_issue(t)
            for t in range(AT):
                if t + ALOOK < AT:
                    a_issue(t + ALOOK)
                a_consume(t)

            loc = small.tile([128, 1], f32, name="loc")
            nc.vector.tensor_reduce(
                loc[:], partials[:], axis=mybir.AxisListType.X, op=add
            )
            # Gate the x/r stream (emitted below, same sync ring) behind the
            # local abs-sum: its transfers otherwise steal fabric bandwidth
            # from the phase-A stream, which is the AllReduce's critical path
            # on every core.
            xgate = small.tile([128, 1], f32, name="xgate")
            nc.sync.dma_start(xgate[:], loc[:])
            # Bounce DMAs + collective all on gpsimd: the in-order queue
            # naturally sequences write -> AllReduce -> readback, and the
            # fp32 W stream below is queued BEHIND the readback on this same
            # ring, keeping the fabric quiet while the collective runs.
            cc_in = dram_pool.tile([128, 1], f32, name="cc_in")
            cc_out = dram_pool.tile([128, 1], f32, name="cc_out", addr_space="Shared")
            nc.gpsimd.dma_start(cc_in[:], loc[:])
            with tc.high_priority():
                nc.gpsimd.collective_compute(
                    "AllReduce",
                    add,
                    replica_groups=[list(range(N_CORES))],
                    ins=[cc_in.opt()],
                    outs=[cc_out.opt()],
                )
            ar_sb = small.tile([128, 1], f32, name="ar_sb")
            nc.gpsimd.dma_start(ar_sb[:], cc_out[:])

            # fp32 W stream for the quantizer, nb-major, on the gpsimd ring
            # (behind the collective readback).
            WLOOK = 4
            wtiles = {}

            def w_issue(i):
                nb, g = divmod(i, QG)
                wst = wstage.tile([128, 4, 512], f32, name="wst", tag="wst")
                nc.gpsimd.dma_start(wst[:], wt[nb, :, 4 * g : 4 * g + 4, :])
                wtiles[i] = wst

            for i in range(WLOOK):
                w_issue(i)

            # Reduce across partitions + broadcast: ones[128,128].T @ ar_sb
            ones = small.tile([128, 128], f32, name="ones")
            nc.vector.memset(ones[:], 1.0)
            psum_s = psum_pool.tile([128, 1], f32, name="psum_s", tag="mm")
            nc.tensor.matmul(psum_s[:], ones[:], ar_sb[:], start=True, stop=True)

            scale_sb = small.tile([128, 1], f32, name="scale_sb")
            nc.vector.tensor_scalar(
                out=scale_sb[:], in0=psum_s[:],
                scalar1=INV_NELEM, scalar2=1e-5, op0=mult, op1=amax,
            )
            thr = small.tile([128, 1], f32, name="thr")
            nc.vector.tensor_scalar(
                out=thr[:], in0=scale_sb[:], scalar1=0.5, scalar2=None, op0=mult
            )
            nthr = small.tile([128, 1], f32, name="nthr")
            nc.vector.tensor_scalar(
                out=nthr[:], in0=scale_sb[:], scalar1=-0.5, scalar2=None, op0=mult
            )

            # ---- Phase B: quantize fp32 W -> wq in {-1,0,1} fp8, nb-major so
            # pass 1 (nb=0) can start after a quarter of W is quantized.
            def w_consume(i):
                nb, g = divmod(i, QG)
                wst = wtiles.pop(i)
                t1 = tmp_pool.tile([128, 4, 512], f32, name="t1", tag="t1")
                # t1 = (w < -T)
                nc.vector.tensor_scalar(
                    out=t1[:], in0=wst[:],
                    scalar1=nthr[:], scalar2=None,
                    op0=mybir.AluOpType.is_lt,
                )
                # wq = (w > T) - t1
                nc.vector.scalar_tensor_tensor(
                    out=wq[nb][:, 4 * g : 4 * g + 4, :],
                    in0=wst[:],
                    scalar=thr[:], in1=t1[:],
                    op0=mybir.AluOpType.is_gt, op1=sub,
                )

            for i in range(NQ):
                if i + WLOOK < NQ:
                    w_issue(i + WLOOK)
                w_consume(i)

            # ---- Phase C: out[mb] = x[mb] @ wq.T, fp8 DoubleRow ----
            def x_issue(mb):
                xst = xstage.tile([128, KT, 128], f8, name="xst", tag="xst")
                nc.sync.dma_start(xst[:, :, :], xs[mb])
                rst = rstage.tile([128, KR, 128], f8, name="rst", tag="rst")
                nc.sync.dma_start(rst[:, :, :], rs[mb])
                return xst, rst

            def lhs_of(xt, j):
                xst, rst = xt
                if j < K2:
                    return xst[:, 2 * j : 2 * j + 2, :]
                jj = j - K2
                return rst[:, 2 * jj : 2 * jj + 2, :]

            def mm(psum, xt, j, nb):
                jj = j if j < K2 else j - K2
                nc.tensor.matmul(
                    psum[:],
                    lhs_of(xt, j),
                    wq[nb][:, 2 * jj : 2 * jj + 2, :],
                    start=(j == 0),
                    stop=(j == NJ - 1),
                    perf_mode=dr,
                )

            def evac(mb, nb, psum):
                ost = ostage.tile([128, 512], bf16, name="ost", tag="ost")
                # out = psum * scale, on ScalarE (has a PSUM port), bf16 store
                nc.scalar.activation(
                    ost[:],
                    psum[:],
                    mybir.ActivationFunctionType.Copy,
                    scale=scale_sb[:],
                )
                nc.scalar.dma_start(
                    out[mb * 128 : (mb + 1) * 128, nb * 512 : (nb + 1) * 512],
                    ost[:],
                )

            def do_block(mb, nbs, xt):
                psums = {
                    nb: psum_pool.tile([128, 512], f32, name=f"ps_{mb}_{nb}", tag="mm")
                    for nb in nbs
                }
                for j in range(NJ):
                    for nb in nbs:
                        mm(psums[nb], xt, j, nb)
                for nb in nbs:
                    evac(mb, nb, psums[nb])

            # Pass 1, wave-interleaved prefix: 8 blocks advance 4 k-chunks at
            # a time so the PE tracks the nb=0 quant stream front.
            xts = {}
            for mb in range(WAVED):
                xts[(0, mb)] = x_issue(mb)
            wpsums = [
                psum_pool.tile([128, 512], f32, name=f"wps_{mb}", tag="mm")
                for mb in range(WAVED)
            ]
            for w0 in range(0, NJ, 4):
                for mb in range(WAVED):
                    for j in range(w0, min(w0 + 4, NJ)):
                        mm(wpsums[mb], xts[(0, mb)], j, 0)
            for mb in range(WAVED):
                xts.pop((0, mb))
                evac(mb, 0, wpsums[mb])

            # Pass 1 remainder + pass 2, with x/r prefetched XLOOK blocks out.
            XLOOK = 4
            schedule = [(0, mb, [0]) for mb in range(WAVED, MB)]
            schedule += [(1, mb, [1, 2, 3]) for mb in range(MB)]
            for idx, (p, mb, nbs) in enumerate(schedule):
                if idx == 0:
                    for p2, mb2, _ in schedule[: XLOOK + 1]:
                        xts[(p2, mb2)] = x_issue(mb2)
                look = idx + XLOOK + 1
                if look < len(schedule):
                    p2, mb2, _ = schedule[look]
                    xts[(p2, mb2)] = x_issue(mb2)
                do_block(mb, nbs, xts.pop((p, mb)))

    nc.compile()
    return nc


def _get_nc():
    global _nc_cache
    if _nc_cache is None:
        _nc_cache = _build_nc()
    return _nc_cache


def kernel(x: np.ndarray, weight: np.ndarray) -> np.ndarray:
    global LAST_EXEC_NS, LAST_RESULTS
    x = np.asarray(x, dtype=np.float32)
    weight = np.asarray(weight, dtype=np.float32)

    nc = _get_nc()

    f8 = ml_dtypes.float8_e4m3

    # x -> stationary tile layout [mb, k(part), kt, m]: per (mb, p) the
    # [kt, m] plane is contiguous, so each m-block loads as one DMA.
    xf = x.reshape(M, K)
    x8 = xf.astype(f8)
    xsh = np.ascontiguousarray(
        x8.reshape(MB, 128, KT, 128).transpose(0, 3, 2, 1)
    )
    # residual of the fp8 cast, itself in fp8, for the first 6/16 of K
    r = (xf - x8.astype(np.float32))[:, : KR * 128].astype(f8)
    rsh = np.ascontiguousarray(r.reshape(MB, 128, KR, 128).transpose(0, 3, 2, 1))

    in_maps = []
    for c in range(N_CORES):
        wsh = weight[c * NL : (c + 1) * NL, :]  # [2048, 4096] fp32
        # quant layout [nb, k(part), kt, n]: per partition the (kt, n) plane
        # is contiguous, so a kt-group loads as one descriptor per partition.
        wtc = np.ascontiguousarray(
            wsh.T.reshape(KT, 128, NB, 512).transpose(2, 1, 0, 3)
        )
        # absum stream: fp8(64*w); any partitioning works for a plain sum
        wac = (wsh * np.float32(64.0)).astype(f8).reshape(AT, 128, 2048)
        in_maps.append({"xs": xsh, "rs": rsh, "wa": wac, "wt": wtc})

    # Warm the per-device PJRT dispatch path (device contexts, executable
    # load machinery) with a trivial sharded op so the real launch below
    # starts the 8 cores with minimal stagger - the in-kernel AllReduce
    # otherwise turns launch skew into idle time on every early core.
    import jax
    from jax.sharding import Mesh, NamedSharding, PartitionSpec

    devs = jax.devices()[:N_CORES]
    wmesh = Mesh(np.asarray(devs), ("core",))
    warm = jax.device_put(
        np.zeros((N_CORES, 128), np.float32),
        NamedSharding(wmesh, PartitionSpec("core")),
    )
    jax.jit(lambda t: t + 1.0)(warm).block_until_ready()

    trace = bool(int(os.environ.get("BASS_KERNEL_TRACE", "0")))
    res = run_bass_kernel_spmd(
        nc, in_maps, core_ids=list(range(N_CORES)), trace=trace
    )
    LAST_EXEC_NS = res.exec_time_ns
    LAST_RESULTS = res

    outs = [np.asarray(res.results[c]["out"]) for c in range(N_CORES)]
    full = (
        np.concatenate(outs, axis=1).astype(np.float32).reshape(B, S, NF)
    )
    return full


# revision 43
# speedup vs baseline: 1.0637x; 1.0301x over previous
"""BitLinear (absmean ternary-quantized linear) on 8 TRN2 NeuronCores.

Strategy (tensor-parallel, column sharding, fp8 DoubleRow matmul):
  - weight [16384, 4096] sharded along out-features: 2048 rows per core.
  - absmean scale is global over W: each core computes a per-partition abs-sum
    of its shard, AllReduce(add) across the 8 cores, then a ones-matmul
    reduces across partitions and broadcasts the global sum. The absum input
    is streamed as fp8(64*w) - 64 maps the kaiming bound into e4m3's normal
    range, RNE keeps the mean unbiased to ~1e-5 relative, and the stream is
    4x smaller than fp32 (the head phase is DMA-bound at ~180 GB/s).
  - quantize: wq = (w > T) - (w < -T) with T = 0.5*scale from the fp32 W
    (equivalent to clip(round(w/scale), -1, 1)); stored as fp8e4 {-1,0,1}
    (exact), unscaled; the fp32 scale is applied in the PSUM->SBUF copy.
  - matmul: fp8e4 DoubleRow perf mode - each instruction contracts K=256
    (two 128-row planes) at double rate: lhsT = x tile [128, 2, 128]
    stationary, rhs = wq slice [128, 2, 512] moving, fp32 PSUM accumulate.
  - precision: e4m3 x alone gives rel err ~0.024 (> 2e-2 gate). A residual
    pass r8 = e4m3(x - e4m3(x)) over the first 6/16 of K cancels that much
    of the quantization noise power -> rel err ~0.019 at 1.375x matmul cost,
    still well ahead of bf16 (2x the PE work of the fp8 main pass).
  - the AllReduce is fabric-sensitive: concurrent bulk DMA inflates its
    latency ~7x. The fp32 W stream for the quantizer therefore rides the
    same gpsimd ring as the collective bounces, queued BEHIND the readback,
    so the fabric is quiet during the collective by construction.
  - pass 1 (nb=0) starts while the quantizer is still streaming: the first
    8 m-blocks are wave-interleaved (4 k-chunks across all 8 blocks per
    wave, PSUM groups held open) so the PE tracks the quant stream front
    instead of serializing behind the last chunk of block 0. Pass 2 runs
    nb=1..3 per m-block (stationary x shared by the three nb matmuls).
  - out is stored bf16 (halves the store traffic; adds ~1e-4 to rel err),
    widened to fp32 on the host.
  - rings: sync = absum share + x/r stream; gpsimd = absum share, collective
    bounces, fp32 W stream; scalar = absum share + out stores. Engines:
    DVE = absum reduces + quant math; Activation = absum Abs+accum share +
    scaled PSUM evacuation.
"""

import os
import sys

import numpy as np

sys.path.insert(0, "/opt/trn_rl_repo")

import ml_dtypes  # noqa: E402

from concourse import bacc, mybir, tile  # noqa: E402
from concourse.bass_utils import run_bass_kernel_spmd  # noqa: E402


def _install_ntff_hook_shim():
    """bass_utils' trace path needs antenv.axon_hooks, which this image's
    antenv lacks. Recreate the boot-time hook against the axon PJRT .so so
    NTFF profiling (HW exec_time_ns) works."""
    import contextlib
    import ctypes
    import types

    try:
        from antenv.axon_hooks import get_axon_ntff_profile_hook  # noqa: F401

        return  # real module present
    except ImportError:
        pass

    so_path = "/opt/axon/libaxon_pjrt.so"
    if not os.path.exists(so_path):
        return
    lib = ctypes.CDLL(so_path)
    if not hasattr(lib, "axon_start_nrt_profile"):
        return
    lib.axon_start_nrt_profile.argtypes = [
        ctypes.POINTER(ctypes.c_int64),
        ctypes.c_size_t,
    ]
    lib.axon_start_nrt_profile.restype = ctypes.c_int64
    lib.axon_stop_nrt_profile.argtypes = [ctypes.c_char_p]
    lib.axon_stop_nrt_profile.restype = ctypes.c_int64

    @contextlib.contextmanager
    def _hook(output_dir, device_ids):
        import jax

        jax.devices()
        if device_ids:
            ids = (ctypes.c_int64 * len(device_ids))(*device_ids)
            rc = lib.axon_start_nrt_profile(ids, len(device_ids))
        else:
            rc = lib.axon_start_nrt_profile(None, 0)
        if rc != 0:
            raise RuntimeError(f"axon_start_nrt_profile rc={rc}")
        try:
            yield
        finally:
            n = lib.axon_stop_nrt_profile(str(output_dir).encode())
            if n < 0:
                raise RuntimeError(f"axon_stop_nrt_profile rc={n}")

    mod = types.ModuleType("antenv.axon_hooks")
    _state = {"hook": _hook}
    mod.set_axon_ntff_profile_hook = lambda h: _state.__setitem__("hook", h)
    mod.get_axon_ntff_profile_hook = lambda: _state["hook"]
    sys.modules["antenv.axon_hooks"] = mod


_install_ntff_hook_shim()

N_CORES = 8
B, S, K, NF = 4, 2048, 4096, 16384
M = B * S  # 8192 tokens
NL = NF // N_CORES  # 2048 out-features per core
KT = K // 128  # 32 contraction subtiles of 128
K2 = KT // 2  # 16 DoubleRow chunks of 256
MB = M // 128  # 64 token blocks
NB = NL // 512  # 4 out-feature chunks of 512
K2R = 6  # residual DoubleRow chunks (first 6/16 of K; rel err ~0.019)
KR = 2 * K2R  # residual subtiles (k < 1536)
NJ = K2 + K2R  # matmuls per (mb, nb) accumulation group (22)
AT = 8  # absum stream tiles [128, 8192] fp8 (8KB descriptors - the DMA
#   rings are descriptor-rate-bound, so fat per-partition rows matter)
ASL = 4  # absum compute slices per tile ([128, 2048] each)
MB2 = MB // 2  # x/r stream in 2-block tiles (8KB descriptors)
QG = KT // 4  # quant quad-groups per nb (8)
NQ = NB * QG  # total quant quad tiles (32)
WAVED = 8  # wave-interleaved m-blocks at the start of pass 1
INV_NELEM = 1.0 / (NF * K * 64.0)  # absum stream carries 64*w

LAST_EXEC_NS = None
LAST_RESULTS = None

_nc_cache = None


def _build_nc():
    f32 = mybir.dt.float32
    bf16 = mybir.dt.bfloat16
    f8 = mybir.dt.float8e4

    nc = bacc.Bacc(
        "TRN2", target_bir_lowering=False, debug=False, num_devices=N_CORES
    )
    xs = nc.declare_dram_parameter("xs", [MB2, 128, 2, KT, 128], f8, isOutput=False)
    rs = nc.declare_dram_parameter("rs", [MB2, 128, 2, KR, 128], f8, isOutput=False)
    wa = nc.declare_dram_parameter("wa", [AT, 128, 8192], f8, isOutput=False)
    wt = nc.declare_dram_parameter("wt", [NB, 128, KT, 512], f32, isOutput=False)
    out = nc.declare_dram_parameter("out", [M, NL], bf16, isOutput=True)

    add = mybir.AluOpType.add
    mult = mybir.AluOpType.mult
    sub = mybir.AluOpType.subtract
    amax = mybir.AluOpType.max
    dr = mybir.MatmulPerfMode.DoubleRow

    with tile.TileContext(nc) as tc:
        with (
            tc.tile_pool(name="wq_pool", bufs=1) as wq_pool,
            tc.tile_pool(name="astage", bufs=3) as astage,
            tc.tile_pool(name="wstage", bufs=4) as wstage,
            tc.tile_pool(name="tmp_pool", bufs=1) as tmp_pool,
            tc.tile_pool(name="xstage", bufs=5) as xstage,
            tc.tile_pool(name="rstage", bufs=5) as rstage,
            tc.tile_pool(name="ostage", bufs=6) as ostage,
            tc.tile_pool(name="small", bufs=1) as small,
            tc.tile_pool(name="psum", bufs=8, space="PSUM") as psum_pool,
            tc.tile_pool(name="dram", bufs=1, space="DRAM") as dram_pool,
        ):
            # Resident quantized weights, one tile per nb so pass-1 reads of
            # the nb=0 tile never false-depend on quant writes to nb=1..3.
            # Each is [128(k), kt, 512], sliced [:, 2j:2j+2, :] as the
            # DoubleRow moving operand.
            wq = {
                nb: wq_pool.tile([128, KT, 512], f8, name=f"wq{nb}", tag=f"wq{nb}")
                for nb in range(NB)
            }

            # ---- Phase A absum: fp8(64*w) streamed round-robin over all
            # three free DMA rings; abs-sums split between DVE
            # (tensor_reduce) and the Activation engine (Abs + accum).
            # Software-pipelined: DMA lookahead stays below the pool depth so
            # ring-buffer reuse always lands behind an already-emitted reader.
            ALOOK = 2
            atiles = {}
            arings = (nc.sync, nc.gpsimd, nc.scalar)

            def a_issue(t):
                wab = astage.tile([128, 8192], f8, name="wab", tag="wab")
                arings[t % 3].dma_start(wab[:], wa[t])
                atiles[t] = wab

            trash = small.tile([128, 2048], bf16, name="trash")
            partials = small.tile([128, AT * ASL], f32, name="partials")

            def a_consume(s):
                # consume slice s of tile s // ASL; alternate DVE / Activation
                wab = atiles[s // ASL]
                sl = s % ASL
                view = wab[:, sl * 2048 : (sl + 1) * 2048]
                if s % 2 == 0:
                    nc.vector.tensor_reduce(
                        partials[:, s : s + 1],
                        view,
                        axis=mybir.AxisListType.X,
                        op=add,
                        apply_absolute_value=True,
                    )
                else:
                    nc.scalar.activation(
                        trash[:],
                        view,
                        mybir.ActivationFunctionType.Abs,
                        accum_out=partials[:, s : s + 1],
                    )

            for t in range(ALOOK):
                a_issue(t)
            for s in range(AT * ASL):
                if s % ASL == 0 and s // ASL + ALOOK < AT:
                    a_issue(s // ASL + ALOOK)
                a_consume(s)
            atiles.clear()

            loc = small.tile([128, 1], f32, name="loc")
            nc.vector.tensor_reduce(
                loc[:], partials[:], axis=mybir.AxisListType.X, op=add
            )
            # Gate the x/r stream (emitted below, same sync ring) behind the
            # local abs-sum: its transfers otherwise steal fabric bandwidth
            # from the phase-A stream, which is the AllReduce's critical path
            # on every core.
            xgate = small.tile([128, 1], f32, name="xgate")
            nc.sync.dma_start(xgate[:], loc[:])
            # Bounce DMAs + collective all on gpsimd: the in-order queue
            # naturally sequences write -> AllReduce -> readback, and the
            # fp32 W stream below is queued BEHIND the readback on this same
            # ring, keeping the fabric quiet while the collective runs.
            cc_in = dram_pool.tile([128, 1], f32, name="cc_in")
            cc_out = dram_pool.tile([128, 1], f32, name="cc_out", addr_space="Shared")
            nc.gpsimd.dma_start(cc_in[:], loc[:])
            with tc.high_priority():
                nc.gpsimd.collective_compute(
                    "AllReduce",
                    add,
                    replica_groups=[list(range(N_CORES))],
                    ins=[cc_in.opt()],
                    outs=[cc_out.opt()],
                )
            ar_sb = small.tile([128, 1], f32, name="ar_sb")
            nc.gpsimd.dma_start(ar_sb[:], cc_out[:])

            # fp32 W stream for the quantizer, nb-major, on the gpsimd ring
            # (behind the collective readback).
            WLOOK = 2
            wtiles = {}

            def w_issue(i):
                nb, g = divmod(i, QG)
                wst = wstage.tile([128, 4, 512], f32, name="wst", tag="wst")
                nc.gpsimd.dma_start(wst[:], wt[nb, :, 4 * g : 4 * g + 4, :])
                wtiles[i] = wst

            for i in range(WLOOK):
                w_issue(i)

            # Reduce across partitions + broadcast: ones[128,128].T @ ar_sb
            ones = small.tile([128, 128], f32, name="ones")
            nc.vector.memset(ones[:], 1.0)
            psum_s = psum_pool.tile([128, 1], f32, name="psum_s", tag="mm")
            nc.tensor.matmul(psum_s[:], ones[:], ar_sb[:], start=True, stop=True)

            scale_sb = small.tile([128, 1], f32, name="scale_sb")
            nc.vector.tensor_scalar(
                out=scale_sb[:], in0=psum_s[:],
                scalar1=INV_NELEM, scalar2=1e-5, op0=mult, op1=amax,
            )
            thr = small.tile([128, 1], f32, name="thr")
            nc.vector.tensor_scalar(
                out=thr[:], in0=scale_sb[:], scalar1=0.5, scalar2=None, op0=mult
            )
            nthr = small.tile([128, 1], f32, name="nthr")
            nc.vector.tensor_scalar(
                out=nthr[:], in0=scale_sb[:], scalar1=-0.5, scalar2=None, op0=mult
            )

            # ---- Phase B: quantize fp32 W -> wq in {-1,0,1} fp8, nb-major so
            # pass 1 (nb=0) can start after a quarter of W is quantized.
            def w_consume(i):
                nb, g = divmod(i, QG)
                wst = wtiles.pop(i)
                t1 = tmp_pool.tile([128, 4, 512], f32, name="t1", tag="t1")
                # t1 = (w < -T)
                nc.vector.tensor_scalar(
                    out=t1[:], in0=wst[:],
                    scalar1=nthr[:], scalar2=None,
                    op0=mybir.AluOpType.is_lt,
                )
                # wq = (w > T) - t1
                nc.vector.scalar_tensor_tensor(
                    out=wq[nb][:, 4 * g : 4 * g + 4, :],
                    in0=wst[:],
                    scalar=thr[:], in1=t1[:],
                    op0=mybir.AluOpType.is_gt, op1=sub,
                )

            for i in range(NQ):
                if i + WLOOK < NQ:
                    w_issue(i + WLOOK)
                w_consume(i)

            # ---- Phase C: out[mb] = x[mb] @ wq.T, fp8 DoubleRow ----
            # x/r stream in 2-block tiles: 8 KiB per-partition rows halve the
            # descriptor count on the (descriptor-rate-bound) sync ring.
            def x_issue(mb2):
                xst = xstage.tile([128, 2, KT, 128], f8, name="xst", tag="xst")
                nc.sync.dma_start(xst[:, :, :, :], xs[mb2])
                rst = rstage.tile([128, 2, KR, 128], f8, name="rst", tag="rst")
                nc.sync.dma_start(rst[:, :, :, :], rs[mb2])
                return xst, rst

            def lhs_of(xt, b, j):
                xst, rst = xt
                if j < K2:
                    return xst[:, b, 2 * j : 2 * j + 2, :]
                jj = j - K2
                return rst[:, b, 2 * jj : 2 * jj + 2, :]

            def mm(psum, xt, b, j, nb):
                jj = j if j < K2 else j - K2
                nc.tensor.matmul(
                    psum[:],
                    lhs_of(xt, b, j),
                    wq[nb][:, 2 * jj : 2 * jj + 2, :],
                    start=(j == 0),
                    stop=(j == NJ - 1),
                    perf_mode=dr,
                )

            def evac(mb, nb, psum):
                ost = ostage.tile([128, 512], bf16, name="ost", tag="ost")
                # out = psum * scale, on ScalarE (has a PSUM port), bf16 store
                nc.scalar.activation(
                    ost[:],
                    psum[:],
                    mybir.ActivationFunctionType.Copy,
                    scale=scale_sb[:],
                )
                nc.scalar.dma_start(
                    out[mb * 128 : (mb + 1) * 128, nb * 512 : (nb + 1) * 512],
                    ost[:],
                )

            def do_block(mb, nbs, xt, b):
                psums = {
                    nb: psum_pool.tile([128, 512], f32, name=f"ps_{mb}_{nb}", tag="mm")
                    for nb in nbs
                }
                for j in range(NJ):
                    for nb in nbs:
                        mm(psums[nb], xt, b, j, nb)
                for nb in nbs:
                    evac(mb, nb, psums[nb])

            # Pass 1, wave-interleaved prefix: 8 blocks advance 4 k-chunks at
            # a time so the PE tracks the nb=0 quant stream front.
            xts = {}
            for t2 in range(WAVED // 2):
                xts[(0, t2)] = x_issue(t2)
            wpsums = [
                psum_pool.tile([128, 512], f32, name=f"wps_{mb}", tag="mm")
                for mb in range(WAVED)
            ]
            for w0 in range(0, NJ, 4):
                for mb in range(WAVED):
                    for j in range(w0, min(w0 + 4, NJ)):
                        mm(wpsums[mb], xts[(0, mb // 2)], mb % 2, j, 0)
            for mb in range(WAVED):
                evac(mb, 0, wpsums[mb])
            for t2 in range(WAVED // 2):
                xts.pop((0, t2))

            # Pass 1 remainder + pass 2, with x/r prefetched XLOOK tiles out.
            XLOOK = 2
            schedule = [(0, t2, [0]) for t2 in range(WAVED // 2, MB2)]
            schedule += [(1, t2, [1, 2, 3]) for t2 in range(MB2)]
            for idx, (p, t2, nbs) in enumerate(schedule):
                if idx == 0:
                    for p2, u2, _ in schedule[: XLOOK + 1]:
                        xts[(p2, u2)] = x_issue(u2)
                look = idx + XLOOK + 1
                if look < len(schedule):
                    p2, u2, _ = schedule[look]
                    xts[(p2, u2)] = x_issue(u2)
                xt = xts.pop((p, t2))
                for b in (0, 1):
                    do_block(2 * t2 + b, nbs, xt, b)

    nc.compile()
    return nc


def _get_nc():
    global _nc_cache
    if _nc_cache is None:
        _nc_cache = _build_nc()
    return _nc_cache


def kernel(x: np.ndarray, weight: np.ndarray) -> np.ndarray:
    global LAST_EXEC_NS, LAST_RESULTS
    x = np.asarray(x, dtype=np.float32)
    weight = np.asarray(weight, dtype=np.float32)

    nc = _get_nc()

    f8 = ml_dtypes.float8_e4m3

    # x -> stationary tile layout [mb, k(part), kt, m]: per (mb, p) the
    # [kt, m] plane is contiguous, so each m-block loads as one DMA.
    xf = x.reshape(M, K)
    x8 = xf.astype(f8)
    # [mb2, k(part), b, kt, m]: 8 KiB contiguous per partition per 2-block tile
    xsh = np.ascontiguousarray(
        x8.reshape(MB2, 2, 128, KT, 128).transpose(0, 4, 1, 3, 2)
    )
    # residual of the fp8 cast, itself in fp8, for the first 6/16 of K
    r = (xf - x8.astype(np.float32))[:, : KR * 128].astype(f8)
    rsh = np.ascontiguousarray(
        r.reshape(MB2, 2, 128, KR, 128).transpose(0, 4, 1, 3, 2)
    )

    in_maps = []
    for c in range(N_CORES):
        wsh = weight[c * NL : (c + 1) * NL, :]  # [2048, 4096] fp32
        # quant layout [nb, k(part), kt, n]: per partition the (kt, n) plane
        # is contiguous, so a kt-group loads as one descriptor per partition.
        wtc = np.ascontiguousarray(
            wsh.T.reshape(KT, 128, NB, 512).transpose(2, 1, 0, 3)
        )
        # absum stream: fp8(64*w); any partitioning works for a plain sum
        wac = (wsh * np.float32(64.0)).astype(f8).reshape(AT, 128, 8192)
        in_maps.append({"xs": xsh, "rs": rsh, "wa": wac, "wt": wtc})

    # Warm the per-device PJRT dispatch path (device contexts, executable
    # load machinery) with a trivial sharded op so the real launch below
    # starts the 8 cores with minimal stagger - the in-kernel AllReduce
    # otherwise turns launch skew into idle time on every early core.
    import jax
    from jax.sharding import Mesh, NamedSharding, PartitionSpec

    devs = jax.devices()[:N_CORES]
    wmesh = Mesh(np.asarray(devs), ("core",))
    warm = jax.device_put(
        np.zeros((N_CORES, 128), np.float32),
        NamedSharding(wmesh, PartitionSpec("core")),
    )
    jax.jit(lambda t: t + 1.0)(warm).block_until_ready()

    trace = bool(int(os.environ.get("BASS_KERNEL_TRACE", "0")))
    res = run_bass_kernel_spmd(
        nc, in_maps, core_ids=list(range(N_CORES)), trace=trace
    )
    LAST_EXEC_NS = res.exec_time_ns
    LAST_RESULTS = res

    outs = [np.asarray(res.results[c]["out"]) for c in range(N_CORES)]
    full = (
        np.concatenate(outs, axis=1).astype(np.float32).reshape(B, S, NF)
    )
    return full
